# revision 43
# baseline (speedup 1.0000x reference)
"""Multi-headed self-attention Trainium2 kernel (8 NeuronCores).

Problem: B=4, S=2048, D=768, H=12 heads of DH=64; fp32 inputs.

Sharding: core c handles batch b = c//2 and head group g = c%2 (6 heads).
Each core gets x[b] pre-transposed to x^T [768, 2048] (host-side layout,
cast fp16), its 384-column slices of Wq/Wk/Wv (fp16) and biases, and
mask[b].

Device (per core):
  Q^T, K^T  [384, 2048] = W-slice.T @ x^T; bias added on the PSUM->SBUF
            evacuation. Heads are packed in pairs: m-block mb holds head
            2mb on partitions 0-63 and head 2mb+1 on partitions 64-127.
  V         [2048, 384] natural; the value bias is algebraically moved to
            the epilogue (out = raw/den + bv, exact because bv factors out
            of the softmax average). The padding mask is folded into V
            multiplicatively; a 65th "ones" column per head makes attn@V
            also produce the softmax denominator.
  scores^T  [128, 512] blocks = K_h Q_h^T; the two heads of an m-block run
            concurrently on the PE via row tiling (K=64, tile_position
            (0,0)/(64,0)). Score tiles are [128, 1024] (2 PSUM banks, one
            2-key-block group), triple buffered: 6 banks + 2 attn@V
            accumulator banks. 8 groups per query stripe.
  attn^T    EVERY group splits across both exp engines concurrently:
            head A's [128, 1024] tile is one exp ACTIVATE on ScalarE
            while head B's runs the Schraudolph fp16 exp on VectorE
            (i16 = f32->i16(scores*(1024*log2e/8) + (15*1024 - 60)),
            bitcast to fp16 == 2^(x*log2e) ~ e^x, ~3% elementwise, 50%
            of the stream -> 7.4e-3 end-to-end). Same engine totals as
            an alternating group split, but the per-group exp latency
            (the scores->exp->attn@V chain link that sets the pipeline
            period and the PSUM-free time) halves to ~1.2us.
  out^T_aug [65, 512] += [V_h | 1].T @ attn^T chunks, accumulated over Sk.
            attn@V for group g is emitted one group later so the next
            group's score matmuls keep the exp engines fed.
  epilogue  out^T = out^T_aug[0:64] / row64 + bv; pipelined across stripes
            in 3 stages (evacuate+store-den / load+recip+store / bcast+
            normalize+store-out). Both heads are stacked into one
            [128, 512] tile (cross-partition-base engine copies verified
            on HW) and the normalize (mult by 1/den bcast + bias add)
            runs on the otherwise-idle GPSIMD engine, freeing the DVE
            for its exp share. The final stripe broadcasts 1/den across
            partitions with a K=2 fp32 PE matmul against a 2-row
            selector (no DRAM round trip) after a stacked 1/den on
            partitions 0+64: magic-constant approx (0x7EF31200) + one
            Newton step, 4 streaming DVE ops (the exact iterative
            RECIPROCAL is 8 cyc/elem = 4.3us). Epilogue DMAs ride the
            GPSIMD SWDGE queue so the sync ring stays short.

Q/K projection chunks and V-projection chunks are emitted through a
slotted task queue interleaved into the attention loop, paced so the PE
runs them in the slack left by the exp engines. Inputs are host-relaid
(xT phase-major so each phase lands as one contiguous [128, 3KB-row]
DMA; weights m-block-major) and spread across all three DMA-capable
queues (SP + Activation + GPSIMD, each ring only sustains ~22-30 GB/s),
first-use-first; the critical wq-mb0/wk-mb0/xT-phase0 slab is split
three ways. ~240 dependency-free warm-up matmuls run in the DMA shadow
so the PE's HAM clock gate reaches 2.4 GHz before real work.

Host gathers: out[b][:, g*384:(g+1)*384] = core_out.T (layout only).
Matmuls run in fp16 (fp32 PSUM accumulate); softmax and normalization are
fp32 except the group-4 fast-exp path.
"""

import heapq

import numpy as np

B, S, D, H = 4, 2048, 768, 12
DH = 64          # head dim
HPC = 6          # heads per core
DHC = HPC * DH   # 384 = per-core slice of D
N_CORES = 8
P = 128
KC = D // P      # 6 contraction chunks
NSK = S // P     # 16 key blocks
NQS = S // 512   # 4 query stripes

GROUPS = tuple((2 * g, 2 * g + 1) for g in range(8))
NG = len(GROUPS)
# fp16 Schraudolph: exp(s/8) ~ bitcast_f16(i16(s * K8 + BEXP))
K8 = 1024.0 * np.log2(np.e) / 8.0
BEXP = 1024.0 * 15 - 60.0

_CACHED = None


def _build_module():
    import concourse.bacc as bacc
    import concourse.tile as tile
    from concourse import mybir

    f32 = mybir.dt.float32
    f16 = mybir.dt.float16
    i16 = mybir.dt.int16
    i32 = mybir.dt.int32
    EXP = mybir.ActivationFunctionType.Exp
    LN = mybir.ActivationFunctionType.Ln
    COPY = mybir.ActivationFunctionType.Copy
    ADD = mybir.AluOpType.add
    MULT = mybir.AluOpType.mult

    nc = bacc.Bacc(trn_type="TRN2")

    # host pre-interleaves the contraction chunks so every input loads as
    # one DMA with large contiguous rows. xT is additionally pre-split into
    # 4 query/key column phases: xT[p, ph, c, 512] so each phase is one DMA
    # with 3 KB contiguous rows, staged in first-use order.
    xT = nc.dram_tensor("xT", [P, NQS, KC * 512], f16, kind="ExternalInput")
    wq = nc.dram_tensor("wq", [P, KC * DHC], f16, kind="ExternalInput")
    wk = nc.dram_tensor("wk", [P, KC * DHC], f16, kind="ExternalInput")
    wv = nc.dram_tensor("wv", [P, KC * DHC], f16, kind="ExternalInput")
    # biases packed [128, 9]: cols 0-2 = bq per m-block, 3-5 = bk, 6-8 = bv
    ball = nc.dram_tensor("ball", [P, 9], f32, kind="ExternalInput")
    maskc = nc.dram_tensor("maskc", [P, NSK], i32, kind="ExternalInput")
    eye2 = nc.dram_tensor("eye2", [65, P], f16, kind="ExternalInput")
    out = nc.dram_tensor("out", [DHC, S], f32, kind="ExternalOutput")

    # per (stripe, head) denominator rows for the reciprocal round-trips
    sums_dram = nc.dram_tensor("sums_scratch", [2 * 12, 512], f32,
                               kind="Internal")
    rec_dram = nc.dram_tensor("rec_scratch", [2 * 12, 512], f32,
                              kind="Internal")
    sums2 = sums_dram.rearrange("e (a b) -> (e a) b", b=P)  # [96, 128]
    rec2 = rec_dram.rearrange("e (a b) -> (e a) b", b=P)
    sums_w = sums_dram.rearrange("(s t) n -> s (t n)", t=2)  # [12, 1024]

    xT4 = xT.rearrange("p f (c n) -> p f c n", n=512)

    with tile.TileContext(nc) as tc:
        sb = tc.alloc_tile_pool(name="sb", bufs=1)
        wk2 = sb  # single SBUF pool: every ring tile passes explicit bufs
        ps = tc.alloc_tile_pool(name="ps", bufs=3, space="PSUM")

        # ---- constants ----
        ball_sb = sb.tile([P, 9], f32)
        nc.sync.dma_start(ball_sb, ball[:, :])
        bq_sb = ball_sb[:, 0:3]
        bk_sb = ball_sb[:, 3:6]
        bv_sb = ball_sb[:, 6:9]
        mask_i = sb.tile([P, NSK], i32)
        nc.sync.dma_start(mask_i, maskc[:, :])
        mask_f = sb.tile([P, NSK], f32)
        nc.vector.tensor_copy(mask_f, mask_i)

        # force the exp ACT_TABLE_LOAD onto the DMA-wait shadow at t=0
        warm = sb.tile([P, 1], f32)
        nc.scalar.activation(warm, ball_sb[:, 0:1], func=EXP)

        ones16 = sb.tile([P, 64], f16)
        nc.vector.memset(ones16, 1.0)
        # 2-row selector for the tail's 1/den partition-broadcast matmul:
        # out rows 0-63 <- rec row 0 (head A), rows 64-127 <- row 64
        # (head B); host constant (memset cannot hit partition base 64
        # alone with zeros elsewhere), loaded on the idle GPSIMD queue
        eye2_sb = sb.tile([65, P], f16)
        nc.gpsimd.dma_start(eye2_sb, eye2[:, :])

        # ---- inputs; split across both HWDGE queues (SP + Activation) by
        # partition halves so the two DMA rings fill SBUF concurrently ----
        # phase-major xT so each phase DMA writes one contiguous
        # [128, 3072] block (the chunk-major layout scattered every
        # phase into 6x1KB chunks per partition -> descriptor-bound DMA)
        xT_sb = sb.tile([P, NQS, KC, 512], f16)
        wq_sb = sb.tile([P, 3, KC, P], f16)
        wk_sb = sb.tile([P, 3, KC, P], f16)
        wv_sb = sb.tile([P, KC, DHC], f16)

        def dma2(dst, src):
            nc.sync.dma_start(dst[0:64], src[0:64])
            nc.scalar.dma_start(dst[64:P], src[64:P])

        # weights are m-block-major host-side so the first projections only
        # wait on their own m-block's slab
        wqv = wq_sb.rearrange("p mb c n -> p (mb c n)")
        wkv = wk_sb.rearrange("p mb c n -> p (mb c n)")
        xT_dst = xT_sb

        # three DMA-capable queues (SP, Activation, GPSIMD); each ring
        # sustains only ~22-30 GB/s, so the critical first slab
        # (wq-mb0 + wk-mb0 + xT phase 0 = 1.18 MB, gates the first
        # projection) is split ~evenly across all three, first-use-first.
        # The Activation queue only carries loads issued before any exp
        # ACTIVATE (later ones head-of-line block the ACTs).
        nc.sync.dma_start(wqv[:, 0:KC * P], wq[:, 0:KC * P])
        nc.scalar.dma_start(wkv[:, 0:KC * P], wk[:, 0:KC * P])
        nc.sync.dma_start(xT_dst[0:32, 0], xT4[0:32, 0])
        nc.scalar.dma_start(xT_dst[32:64, 0], xT4[32:64, 0])
        nc.gpsimd.dma_start(xT_dst[64:P, 0], xT4[64:P, 0])
        dma2(xT_dst[:, 1], xT4[:, 1])
        nc.gpsimd.dma_start(wv_sb.rearrange("p c n -> p (c n)"), wv[:, :])
        nc.gpsimd.dma_start(xT_dst[:, 2], xT4[:, 2])
        nc.sync.dma_start(wqv[:, KC * P:], wq[:, KC * P:])
        nc.sync.dma_start(wkv[:, KC * P:], wk[:, KC * P:])
        nc.gpsimd.dma_start(xT_dst[:, 3], xT4[:, 3])

        # warm the PE's HAM clock gate in the DMA shadow: ~30 dependency-free
        # matmuls keep the array busy so the first real projections run at
        # 2.4 GHz instead of the cold 1.2 GHz default
        warm_ps = ps.tile([64, 64], f32, tag="sc", name="warm_ps")
        for i in range(240):
            nc.tensor.matmul(warm_ps, ones16[:, 0:64], ones16[:, 0:64],
                             start=True, stop=True)

        # ---- persistent activations ----
        QT_sb = sb.tile([P, 3, S], f16)
        KT_sb = sb.tile([P, 3, S], f16)
        V_sb = sb.tile([P, NSK, HPC * 65], f16)
        V_sb4 = V_sb.rearrange("p n (h e) -> p n h e", e=65)

        def emit_qk_proj(dst, w_sb, b_sb, mb, cols):
            """1-2 [128, 512] chunks of Q^T or K^T (heads 2mb, 2mb+1).

            Paired chunks share the per-c stationary weights back-to-back so
            codegen can skip the redundant LDWEIGHTS."""
            pps = ps.tile([P, 1024], f32, tag="sc", name="pps")
            for c in range(KC):
                for i, col0 in enumerate(cols):
                    nc.tensor.matmul(
                        pps[:, i * 512:(i + 1) * 512],
                        w_sb[:, mb, c, :],
                        xT_sb[:, col0 // 512, c, :],
                        start=(c == 0), stop=(c == KC - 1),
                    )
            if len(cols) == 2 and cols[1] == cols[0] + 512:
                # contiguous column pair: one bias-add evacuation
                nc.vector.tensor_scalar(
                    dst[:, mb, cols[0]:cols[0] + 1024], pps,
                    b_sb[:, mb:mb + 1], None, ADD,
                )
            else:
                for i, col0 in enumerate(cols):
                    nc.vector.tensor_scalar(
                        dst[:, mb, col0:col0 + 512],
                        pps[:, i * 512:(i + 1) * 512],
                        b_sb[:, mb:mb + 1], None, ADD,
                    )

        def emit_v_proj_chunk(sk):
            vps = ps.tile([P, DHC], f32, tag="sc", name="vps")
            for c in range(KC):
                nc.tensor.matmul(
                    vps,
                    xT_sb[:, sk // 4, c, (sk % 4) * P:(sk % 4 + 1) * P],
                    wv_sb[:, c, :],
                    start=(c == 0), stop=(c == KC - 1),
                )
            # evac with the multiplicative mask; fp32 -> fp16. Alternates
            # ScalarE (copy with per-partition scale) and DVE so neither
            # engine is oversubscribed during stripe 0 where all 16 V
            # chunks are built.
            if sk % 2 == 0:
                nc.scalar.activation(
                    V_sb4[:, sk, :, 0:64],
                    vps.rearrange("p (h e) -> p h e", e=64),
                    func=COPY, scale=mask_f[:, sk:sk + 1],
                )
            else:
                nc.vector.tensor_scalar(
                    V_sb4[:, sk, :, 0:64],
                    vps.rearrange("p (h e) -> p h e", e=64),
                    mask_f[:, sk:sk + 1], None, MULT,
                )
            nc.vector.tensor_copy(
                V_sb4[:, sk, :, 64],
                mask_f[:, sk:sk + 1].to_broadcast([P, HPC]),
            )

        # ---- slotted task queue (proj chunks, V chunks, epilogue stages) ----
        tasks = []
        seq_counter = [0]

        def add_task(due, fn):
            heapq.heappush(tasks, (due, seq_counter[0], fn))
            seq_counter[0] += 1

        def pump(t):
            while tasks and tasks[0][0] <= t:
                heapq.heappop(tasks)[2]()

        def q_task(mb, qss):
            return lambda: emit_qk_proj(QT_sb, wq_sb, bq_sb, mb,
                                        [qs * 512 for qs in qss])

        def k_task(mb, chs):
            return lambda: emit_qk_proj(KT_sb, wk_sb, bk_sb, mb,
                                        [ch * 512 for ch in chs])

        # stripe 0 JIT: K cols ch*512 feed groups 2ch,2ch+1; V(kb) at slot
        # kb//2 (just before its first attn@V use, one group lagged)
        add_task(1, k_task(0, (1, 2)))
        add_task(4, k_task(0, (3,)))
        for kb in range(NSK):
            add_task(kb // 2, lambda kb=kb: emit_v_proj_chunk(kb))
        add_task(3, q_task(0, (1,)))
        add_task(8, q_task(0, (2, 3)))
        for mb, base in ((1, 16), (2, 34)):
            add_task(base + 1, k_task(mb, (0, 1)))
            add_task(base + 4, k_task(mb, (2, 3)))
            add_task(base + 7, q_task(mb, (0, 1)))
            add_task(base + 10, q_task(mb, (2, 3)))

        # ---- epilogue pipeline (stacked [128, 512]: head A rows 0-63,
        # head B rows 64-127; normalize runs on the idle GPSIMD engine) ----
        def epilogue_stage_a(s, o_psA, o_psB, holder):
            def fn():
                o_raw = wk2.tile([P, 512], f32, tag="oraw", bufs=4,
                                 name="o_raw")
                den2 = wk2.tile([65, 1024], f32, tag="den2", bufs=2,
                                name="den2")
                nc.vector.tensor_copy(o_raw[0:64], o_psA[0:64, :])
                nc.vector.tensor_copy(o_raw[64:P], o_psB[0:64, :])
                # den rows: DMA has no PSUM route, so hop via SBUF (same
                # partition 64; single-lane copies ride the idler ScalarE),
                # then ONE store for both heads (adjacent DRAM rows)
                nc.scalar.copy(den2[64:65, 0:512], o_psA[64:65, :])
                nc.scalar.copy(den2[64:65, 512:1024], o_psB[64:65, :])
                nc.sync.dma_start(sums_w[s:s + 1, :], den2[64:65, 0:1024])
                holder["raw"] = o_raw
            return fn

        def epilogue_stage_b(s):
            def fn():
                den8 = wk2.tile([8, P], f32, tag="den8", bufs=2, name="den8")
                nc.sync.dma_start(den8, sums2[8 * s:8 * s + 8, :])
                nc.vector.reciprocal_approx_fast(den8, den8)
                # store rides the idle GPSIMD queue: keeps the sync ring
                # short so the den8 load above never queues behind stores
                nc.gpsimd.dma_start(rec2[8 * s:8 * s + 8, :], den8)
            return fn

        def epilogue_stage_c(s, holder):
            mb, qs = divmod(s, 4)
            col = qs * 512

            def fn():
                den_bc = wk2.tile([P, 512], f32, tag="denbc", bufs=4,
                                  name="den_bc")
                nc.gpsimd.dma_start(
                    den_bc[0:64],
                    rec_dram[2 * s:2 * s + 1, :].to_broadcast([64, 512]))
                nc.gpsimd.dma_start(
                    den_bc[64:P],
                    rec_dram[2 * s + 1:2 * s + 2, :].to_broadcast([64, 512]))
                o_raw = holder.pop("raw")
                o_fin = wk2.tile([P, 512], f32, tag="ofin", bufs=4,
                                 name="o_fin")
                nc.gpsimd.tensor_mul(o_fin, o_raw, den_bc)
                # NOTE: gpsimd tensor_scalar with a per-partition AP operand
                # is ~6x slower than tensor_tensor (7.5us vs 1.3us per
                # [128,512]) — use a stride-0 broadcast AP instead
                nc.gpsimd.tensor_tensor(
                    o_fin, o_fin, bv_sb[:, mb:mb + 1].to_broadcast([P, 512]),
                    ADD)
                row = mb * P
                nc.gpsimd.dma_start(out[row:row + P, col:col + 512], o_fin)
            return fn

        def epilogue_tail(s, o_psA, o_psB):
            mb, qs = divmod(s, 4)
            col = qs * 512
            # Stack both heads' denominators on partitions 0 and 64
            # (engine partition bases must be quadrant-aligned). The exact
            # iterative RECIPROCAL costs 8 cycles/elem (4.3us); instead:
            # magic-constant bit-trick approx (5% err) + one Newton step
            # (0.26% max err on the den range) = 4 streaming DVE ops.
            # In-between lanes hold memset 1.0 so everything stays finite;
            # the selector's zero rows drop them in the broadcast matmul.
            den2t = wk2.tile([65, 512], f32, tag="rec2", bufs=1, name="den2t")
            r0t = wk2.tile([65, 512], f32, tag="rec2b", bufs=1, name="r0t")
            e1t = wk2.tile([65, 512], f32, tag="rec2c", bufs=1, name="e1t")
            rec2t = wk2.tile([65, 512], f16, tag="rec2d", bufs=1,
                             name="rec2t")
            nc.vector.memset(den2t[0:64, :], 1.0)
            # one stacking copy per engine so they run in parallel (the
            # Newton chain below waits on both rows)
            nc.scalar.copy(den2t[0:1, :], o_psA[64:65, :])
            nc.vector.tensor_copy(den2t[64:65, :], o_psB[64:65, :])
            nc.vector.tensor_scalar(r0t.bitcast(i32), den2t.bitcast(i32),
                                    -1, 0x7EF31200, MULT, ADD)
            nc.vector.tensor_mul(e1t, den2t, r0t)
            nc.vector.tensor_scalar(e1t, e1t, -1.0, 2.0, MULT, ADD)
            nc.vector.tensor_mul(rec2t, r0t, e1t)
            o_raw = wk2.tile([P, 512], f32, tag="oraw", bufs=4, name="o_rawT")
            nc.scalar.copy(o_raw[0:64], o_psA[0:64, :])
            nc.scalar.copy(o_raw[64:P], o_psB[0:64, :])
            # broadcast 1/den across partitions: K=2 matmul against the
            # 2-row selector (rows 0-63 <- recA, 64-127 <- recB)
            rec_bc = ps.tile([P, 512], f32, tag="sc", name="rec_bc")
            nc.tensor.matmul(rec_bc, eye2_sb, rec2t, start=True, stop=True)
            o_fin = wk2.tile([P, 512], f32, tag="ofin", bufs=4, name="o_finT")
            row = mb * P
            # halved finish: the first half's store overlaps the second
            # half's normalize
            for hf in (0, 1):
                sl = slice(hf * 256, (hf + 1) * 256)
                nc.vector.tensor_mul(o_fin[:, sl], o_raw[:, sl],
                                     rec_bc[:, sl])
                nc.vector.tensor_scalar_add(
                    o_fin[:, sl], o_fin[:, sl], bv_sb[:, mb:mb + 1])
                eng = nc.sync if hf == 0 else nc.scalar
                eng.dma_start(
                    out[row:row + P, col + hf * 256:col + (hf + 1) * 256],
                    o_fin[:, sl])

        # ---- main attention pipeline ----
        attn_tiles = {}
        o_ps_map = {}

        def emit_scores(s, g):
            mb, qs = divmod(s, 4)
            col = qs * 512
            kbs = GROUPS[g]
            n = len(kbs) * 512
            gA = ps.tile([P, n], f32, tag="sc", name="gA")
            gB = ps.tile([P, n], f32, tag="sc", name="gB")
            for j, kb in enumerate(kbs):
                nc.tensor.matmul(
                    gA[:, j * 512:(j + 1) * 512],
                    KT_sb[0:64, mb, kb * P:(kb + 1) * P],
                    QT_sb[0:64, mb, col:col + 512],
                    start=True, stop=True, tile_position=(0, 0),
                )
                nc.tensor.matmul(
                    gB[:, j * 512:(j + 1) * 512],
                    KT_sb[64:P, mb, kb * P:(kb + 1) * P],
                    QT_sb[64:P, mb, col:col + 512],
                    start=True, stop=True, tile_position=(64, 0),
                )
            return gA, gB

        def emit_exp(s, g, gA, gB):
            """Head A's tile on ScalarE ACTIVATE, head B's on the DVE
            Schraudolph — EVERY group. Same engine totals as a 4/4
            group-level split, but the two tiles run CONCURRENTLY, so the
            per-group exp latency (the scores->exp->attn@V chain link that
            sets the pipeline period, and the time until the score PSUM
            banks free) halves from ~2.4us to ~1.2us."""
            n = len(GROUPS[g]) * 512
            attnA = wk2.tile([P, n], f16, tag="attnA", bufs=6, name="attnA")
            attnB = wk2.tile([P, n], f16, tag="attnB", bufs=6, name="attnB")
            if s == 11 and g >= 6:
                # end-of-kernel drain: halve the ACTs so attn@V (and the
                # epilogue behind it) starts after 512 columns, not 1024
                for hf in (0, 1):
                    sl = slice(hf * 512, (hf + 1) * 512)
                    nc.scalar.activation(attnA[:, sl], gA[:, sl],
                                         func=EXP, scale=0.125)
            else:
                nc.scalar.activation(attnA, gA, func=EXP, scale=0.125)
            nc.vector.tensor_scalar(
                attnB.bitcast(i16), gB, K8, BEXP, MULT, ADD)
            attn_tiles[(s, g)] = (attnA, attnB)

        def emit_attnv_iter(s, g):
            """Generator: yields after each attn@V matmul so the caller can
            interleave them between the next group's score matmuls. The
            interleave spaces same-PSUM-bank accumulations 4 instructions
            apart (immediate same-bank accumulation costs ~+200 cycles of
            turnaround on the PE)."""
            mb = s // 4
            hA, hB = 2 * mb, 2 * mb + 1
            attnA, attnB = attn_tiles.pop((s, g))
            if s not in o_ps_map:
                o_psA = ps.tile([65, 512], f32, tag="outp", bufs=2,
                                name="o_psA")
                o_psB = ps.tile([65, 512], f32, tag="outp", bufs=2,
                                name="o_psB")
                o_ps_map[s] = (o_psA, o_psB)
            o_psA, o_psB = o_ps_map[s]
            kbs = GROUPS[g]
            for j, kb in enumerate(kbs):
                st = g == 0 and j == 0
                sp = g == NG - 1 and j == len(kbs) - 1
                nc.tensor.matmul(
                    o_psA,
                    V_sb[:, kb, hA * 65:(hA + 1) * 65],
                    attnA[:, j * 512:(j + 1) * 512],
                    start=st, stop=sp,
                )
                yield
                nc.tensor.matmul(
                    o_psB,
                    V_sb[:, kb, hB * 65:(hB + 1) * 65],
                    attnB[:, j * 512:(j + 1) * 512],
                    start=st, stop=sp,
                )
                yield
            if g == NG - 1:
                o_ps_map.pop(s)
                if s == 11:
                    # tail: latency-critical, no DRAM round trip — broadcast
                    # 1/den across partitions with a PE matmul instead
                    epilogue_tail(s, o_psA, o_psB)
                else:
                    holder = {}
                    t_end = NG * (s + 1)
                    add_task(t_end + 1,
                             epilogue_stage_a(s, o_psA, o_psB, holder))
                    # stripe 10's normalize is deferred past the tail's
                    # DVE Newton chain (GPSIMD shares the DVE SBUF port)
                    off_b, off_c = (4, 7) if s == 10 else (4, 6)
                    add_task(t_end + off_b, epilogue_stage_b(s))
                    add_task(t_end + off_c, epilogue_stage_c(s, holder))

        # prologue: first stripe's first Q chunk + first K chunk
        emit_qk_proj(QT_sb, wq_sb, bq_sb, 0, [0])
        emit_qk_proj(KT_sb, wk_sb, bk_sb, 0, [0])

        # NOTE: interleaving attn@V matmuls between score pairs was tried
        # and REGRESSED (+15us): it forces 4 row-tiling mode switches per
        # group (each drains the PE array) and breaks score-pair overlap.
        # Keep scores and attn@V as contiguous blocks (2 switches/group).
        for s in range(12):
            for g in range(NG):
                t = NG * s + g
                gA, gB = emit_scores(s, g)
                emit_exp(s, g, gA, gB)
                pump(t)
                if g >= 1:
                    av = emit_attnv_iter(s, g - 1)
                elif s >= 1:
                    av = emit_attnv_iter(s - 1, NG - 1)
                else:
                    av = None
                if av is not None:
                    for _ in av:
                        pass
        for _ in emit_attnv_iter(11, NG - 1):
            pass
        pump(1 << 30)

        assert not tasks and not attn_tiles and not o_ps_map

        ps.release()
        sb.release()

    nc.finalize()
    return nc


def _get_module():
    global _CACHED
    if _CACHED is None:
        _CACHED = _build_module()
    return _CACHED


def kernel(x, mask, Wq, bq, Wk, bk, Wv, bv):
    from concourse.bass_utils import run_bass_kernel_spmd

    x = np.asarray(x, dtype=np.float32)
    mask = np.asarray(mask, dtype=np.int32)
    Wq = np.asarray(Wq, dtype=np.float32)
    Wk = np.asarray(Wk, dtype=np.float32)
    Wv = np.asarray(Wv, dtype=np.float32)
    bq = np.asarray(bq, dtype=np.float32)
    bk = np.asarray(bk, dtype=np.float32)
    bv = np.asarray(bv, dtype=np.float32)

    nc = _get_module()

    def chunk_rows(a):
        # [768, n] -> [128, 6*n]: row p = [chunk0 | ... | chunk5] slabs so
        # the device loads it as one large-contiguous-row DMA
        n = a.shape[1]
        return np.ascontiguousarray(
            a.reshape(KC, P, n).transpose(1, 0, 2).reshape(P, KC * n))

    def xt_phases(a):
        # [768, 2048] -> [128, 4, 6*512]: phase-major, chunk-interleaved
        return np.ascontiguousarray(
            a.reshape(KC, P, NQS, 512).transpose(1, 2, 0, 3)
            .reshape(P, NQS, KC * 512))

    def chunk_rows_mb(a):
        # [768, 384] -> [128, 3*6*128]: m-block-major, then chunk-major,
        # so the first projections only wait on the leading mb0 slab
        return np.concatenate(
            [chunk_rows(a[:, m * P:(m + 1) * P]) for m in range(3)], axis=1)

    xTs = [xt_phases(x[b].T.astype(np.float16)) for b in range(B)]
    maskcs = [np.ascontiguousarray(mask[b].reshape(NSK, P).T) for b in range(B)]
    eye2 = np.zeros((65, P), dtype=np.float16)
    eye2[0, 0:64] = 1.0
    eye2[64, 64:P] = 1.0

    in_maps = []
    for c in range(N_CORES):
        b, g = divmod(c, 2)
        sl = slice(g * DHC, (g + 1) * DHC)
        ball = np.concatenate([
            bq[sl].reshape(3, P).T, bk[sl].reshape(3, P).T,
            bv[sl].reshape(3, P).T], axis=1)
        in_maps.append({
            "xT": xTs[b],
            "wq": chunk_rows_mb(Wq[:, sl].astype(np.float16)),
            "wk": chunk_rows_mb(Wk[:, sl].astype(np.float16)),
            "wv": chunk_rows(Wv[:, sl].astype(np.float16)),
            "ball": np.ascontiguousarray(ball.astype(np.float32)),
            "maskc": maskcs[b],
            "eye2": eye2,
        })

    res = run_bass_kernel_spmd(nc, in_maps, core_ids=list(range(N_CORES)))

    full = np.empty((B, S, D), dtype=np.float32)
    for c in range(N_CORES):
        b, g = divmod(c, 2)
        full[b, :, g * DHC:(g + 1) * DHC] = res.results[c]["out"].T
    return full

